# revision 14
# baseline (speedup 1.0000x reference)
"""Llama4 MoE layer (top-1 routing) as an 8-core Trainium2 kernel.

Sharding: tensor-parallel over the intermediate dim I (TP=8). Every core
processes ALL T=8192 tokens, sorted by their routed expert, but holds only a
512-wide I-slice of every expert's gate/up/down weights. Because the sorted
token stream and the expert boundaries are identical on every core, one SPMD
program serves all 8 cores with zero capacity padding — per-core matmul work
is exactly T * (I/8), immune to expert load imbalance (the expert-parallel
alternative pads every core to max_e count(e)). Weight bytes per core are
unchanged vs expert-parallel (each core still holds 1/8 of all weights).

Device math per core (tokens grouped by expert e with c_e tokens each):
    g[I/8, c_e] = gate_w_slice_eᵀ·Xᵀ, u likewise   (K = H, 16 k-tiles)
    a = silu(g) * u                                 (bf16, SBUF-resident)
    y_partial[H, c_e] = down_w_sliceᵀ·a             (K = I/8, 4 k-tiles)
Partial outputs (bf16) are summed across the 8 cores on the host and
scattered back to the original token order. Matmuls in bf16, PSUM f32.

Schedule: phase A runs m-outer (each gate/up weight stripe sweeps all token
chunks) so weight-DMA demand is spread at ~70 GB/s instead of front-loaded.
Input DMAs issue on the sync HWDGE queue in need-order (x of e+1, dw of e,
gate/up of e+1) — DRAM sources are always ready, so the input stream runs a
full expert ahead of the PE. Output rows accumulate per down-m-tile into one
bf16 staging tile and leave as a single DMA per (expert, m), alternating
between the sync and scalar HWDGE queues (trigger cost is ~0.6 µs per DMA
independent of size, so fewer/bigger transfers win).
"""

import numpy as np
import ml_dtypes

import concourse.bass as bass
import concourse.mybir as mybir
import concourse.tile as tile
from concourse import bacc
from concourse.bass_utils import run_bass_kernel_spmd

SEQ, BS, H, I, E = 2048, 4, 2048, 4096, 8
N_CORES = 8
P = 128
T = SEQ * BS          # 8192 tokens
IS = I // N_CORES     # 512-wide I-slice per core
NKT = H // P          # 16 k-tiles for gate/up (K = H)
NMT = IS // P         # 4 m-tiles for gate/up / k-tiles for down
NCHUNK = 512          # max matmul moving-dim / PSUM bank width (f32)

BF16 = mybir.dt.bfloat16
F32 = mybir.dt.float32
np_bf16 = ml_dtypes.bfloat16

# Stash of the last BassKernelResults (exec_time_ns when BASS_TRACE=1).
last_results = None


def _chunks(c, first_small=False, last_small=False):
    """Split c tokens into near-equal chunks of at most NCHUNK."""
    if c <= 0:
        return []
    out = []
    tail = []
    if first_small and c > 384:
        # Small opening chunk so the first matmuls' DMA deps are tiny.
        out.append(128)
        c -= 128
    if last_small and c > 384:
        # Small closing chunk so the final drain after the last matmul is
        # short (CAST + out DMA of 128 tokens, not 512).
        tail.append(128)
        c -= 128
    n = -(-c // NCHUNK)
    base, rem = divmod(c, n)
    out.extend([base + 1] * rem + [base] * (n - rem))
    return out + tail


def _build(counts):
    """Build + compile the per-core TP-over-I program.

    counts: per-expert token counts (global, identical on all cores).
    """
    nc = bacc.Bacc("TRN2", target_bir_lowering=False, debug=False)
    xt_d = nc.dram_tensor("xt", [P, NKT, T], BF16, kind="ExternalInput")
    gw_d = nc.dram_tensor("gw", [E, NMT, P, NKT, P], BF16, kind="ExternalInput")
    uw_d = nc.dram_tensor("uw", [E, NMT, P, NKT, P], BF16, kind="ExternalInput")
    dw_d = nc.dram_tensor("dw", [E, P, NKT, NMT, P], BF16, kind="ExternalInput")
    out_d = nc.dram_tensor("out", [NKT, P, T], BF16, kind="ExternalOutput")

    silu = mybir.ActivationFunctionType.Silu

    # Process experts smallest-first: the first expert's x arrives during the
    # post-reset DMA bandwidth ramp, so give it the smallest working set.
    order = sorted(range(E), key=lambda e: counts[e])

    # Per-expert chunk lists: (global token offset, local offset, size).
    cum = [0] * (E + 1)
    for e in range(E):
        cum[e + 1] = cum[e] + counts[e]
    plans = []
    off = []
    for i, e in enumerate(order):
        ch = []
        loc = 0
        for sz in _chunks(counts[e], first_small=(i == 0),
                          last_small=(i == E - 1)):
            ch.append((cum[e] + loc, loc, sz))
            loc += sz
        plans.append(ch)
        off.append(cum[e])
    ocounts = [counts[e] for e in order]
    amax = max(counts)

    with tile.TileContext(nc) as tc:
        with (
            tc.tile_pool(name="xp", bufs=2) as xp,
            tc.tile_pool(name="wp", bufs=6) as wp,
            tc.tile_pool(name="dp", bufs=2) as dp,
            tc.tile_pool(name="ap", bufs=2) as ap,
            tc.tile_pool(name="sp", bufs=4) as sp,
            tc.tile_pool(name="op", bufs=8) as op,
            tc.tile_pool(name="pp", bufs=8, space="PSUM") as pp,
        ):
            # Warm the PE HAM clock gate with dummy matmuls while the first
            # DMAs are in flight (no data deps — zeros in, discarded out).
            warm = sp.tile([P, NCHUNK], BF16, tag="warm", bufs=1)
            nc.gpsimd.memset(warm[:], 0.0)
            warm_ps = pp.tile([P, NCHUNK], F32, tag="ps")
            for _ in range(32):
                nc.tensor.matmul(
                    warm_ps[:], warm[:, :P], warm[:], start=True, stop=True
                )

            gw_t = {}
            uw_t = {}
            # First-expert startup, in PE need-order: gate/up m0 stripes, the
            # small first x chunk, then the remaining m stripes and x chunks.
            e0 = order[0]
            gw_t[(0, 0)] = wp.tile([P, NKT, P], BF16, tag="gw", name="gw_0_0")
            nc.sync.dma_start(gw_t[(0, 0)][:], gw_d[e0, 0])
            xe0 = xp.tile([P, NKT, amax], BF16, tag="xt", name="xe0")
            goff0, loc0, sz0 = plans[0][0]
            nc.sync.dma_start(
                xe0[:, :, loc0:loc0 + sz0], xt_d[:, :, goff0:goff0 + sz0]
            )
            uw_t[(0, 0)] = wp.tile([P, NKT, P], BF16, tag="uw", name="uw_0_0")
            nc.sync.dma_start(uw_t[(0, 0)][:], uw_d[e0, 0])
            for goff, loc, sz in plans[0][1:]:
                nc.sync.dma_start(
                    xe0[:, :, loc:loc + sz], xt_d[:, :, goff:goff + sz]
                )
            for m in range(1, NMT):
                gw_t[(0, m)] = wp.tile([P, NKT, P], BF16, tag="gw",
                                       name=f"gw_0_{m}")
                nc.sync.dma_start(gw_t[(0, m)][:], gw_d[e0, m])
                uw_t[(0, m)] = wp.tile([P, NKT, P], BF16, tag="uw",
                                       name=f"uw_0_{m}")
                nc.sync.dma_start(uw_t[(0, m)][:], uw_d[e0, m])

            xe = xe0
            for e in range(E):
                oe = order[e]
                chunks = plans[e]
                c_e = ocounts[e]

                # -- input DMAs for the future, in PE need-order --
                if e + 1 < E:
                    xe_next = xp.tile([P, NKT, amax], BF16, tag="xt",
                                      name=f"xe{e + 1}")
                    nc.sync.dma_start(
                        xe_next[:, :, :ocounts[e + 1]],
                        xt_d[:, :, off[e + 1]:off[e + 1] + ocounts[e + 1]],
                    )
                dw_t = dp.tile([P, NKT, NMT, P], BF16, tag="dw")
                nc.sync.dma_start(dw_t[:], dw_d[oe])
                if e + 1 < E:
                    on = order[e + 1]
                    for m in range(NMT):
                        gw_t[(e + 1, m)] = wp.tile(
                            [P, NKT, P], BF16, tag="gw", name=f"gw_{e+1}_{m}")
                        nc.sync.dma_start(gw_t[(e + 1, m)][:], gw_d[on, m])
                        uw_t[(e + 1, m)] = wp.tile(
                            [P, NKT, P], BF16, tag="uw", name=f"uw_{e+1}_{m}")
                        nc.sync.dma_start(uw_t[(e + 1, m)][:], uw_d[on, m])

                act = ap.tile([P, NMT, amax], BF16, tag="act")

                # ---- phase A (m-outer): gate/up + silu*up ----
                for m in range(NMT):
                    for ci, (goff, loc, sz) in enumerate(chunks):
                        psg = pp.tile([P, sz], F32, tag="ps",
                                      name=f"psg{e}_{ci}_{m}")
                        for k in range(NKT):
                            nc.tensor.matmul(
                                psg[:],
                                gw_t[(e, m)][:, k, :],
                                xe[:, k, loc:loc + sz],
                                start=(k == 0),
                                stop=(k == NKT - 1),
                            )
                        psu = pp.tile([P, sz], F32, tag="ps",
                                      name=f"psu{e}_{ci}_{m}")
                        for k in range(NKT):
                            nc.tensor.matmul(
                                psu[:],
                                uw_t[(e, m)][:, k, :],
                                xe[:, k, loc:loc + sz],
                                start=(k == 0),
                                stop=(k == NKT - 1),
                            )
                        sil = sp.tile([P, NCHUNK], F32, tag="sil")
                        nc.scalar.activation(sil[:, :sz], psg[:], silu)
                        nc.vector.tensor_mul(
                            act[:, m, loc:loc + sz], sil[:, :sz], psu[:]
                        )

                # ---- phase B: down partial sums, one out DMA per m ----
                for m in range(NKT):
                    ot = op.tile([P, amax], BF16, tag="ot")
                    for ci, (goff, loc, sz) in enumerate(chunks):
                        psd = pp.tile([P, sz], F32, tag="ps",
                                      name=f"psd{e}_{ci}_{m}")
                        for k in range(NMT):
                            nc.tensor.matmul(
                                psd[:],
                                dw_t[:, m, k, :],
                                act[:, k, loc:loc + sz],
                                start=(k == 0),
                                stop=(k == NMT - 1),
                            )
                        if e == E - 1 or m % 2 == 0:
                            nc.vector.tensor_copy(ot[:, loc:loc + sz], psd[:])
                        else:
                            nc.scalar.activation(
                                ot[:, loc:loc + sz], psd[:],
                                mybir.ActivationFunctionType.Copy,
                            )
                        if e == E - 1:
                            # Last expert: drain per chunk (CAST on vector,
                            # trigger on sync) so the final out rows leave
                            # right behind the last matmuls instead of
                            # serializing on one engine after the barrier.
                            nc.sync.dma_start(
                                out_d[m][:, goff:goff + sz],
                                ot[:, loc:loc + sz],
                            )
                    if e < E - 1:
                        eng = nc.sync if m % 2 == 0 else nc.scalar
                        eng.dma_start(
                            out_d[m][:, off[e]:off[e] + c_e], ot[:, :c_e]
                        )

                xe = xe_next if e + 1 < E else None

            # Keep the PE (and the HAM activity monitor) busy while the last
            # outputs drain and the exit barrier runs — an idle PE drops the
            # clock to half rate and dilates the epilogue.
            warm_ps2 = pp.tile([P, NCHUNK], F32, tag="ps", name="warm_ps2")
            for _ in range(20):
                nc.tensor.matmul(
                    warm_ps2[:], warm[:, :P], warm[:], start=True, stop=True
                )

    nc.compile()
    return nc


def kernel(hidden_states, router_w, gate_w, up_w, down_w):
    global last_results
    X = np.asarray(hidden_states, dtype=np.float32).reshape(-1, H)
    router_w = np.asarray(router_w, dtype=np.float32)
    gate_w = np.asarray(gate_w, dtype=np.float32)
    up_w = np.asarray(up_w, dtype=np.float32)
    down_w = np.asarray(down_w, dtype=np.float32)

    # --- routing (host): top-1 expert per token, mirrored op-for-op on the
    # reference (jnp.einsum + argmax) so near-tied logits resolve identically.
    import jax.numpy as jnp

    logits = jnp.einsum(
        "sbh,he->sbe", np.asarray(hidden_states, dtype=np.float32), router_w
    )
    eid = np.asarray(jnp.argmax(logits, axis=-1)).reshape(-1)  # [T]
    perm = np.argsort(eid, kind="stable")
    counts = np.bincount(eid, minlength=E).tolist()

    Xs = X[perm]  # [T, H] sorted by expert
    xt = (
        np.ascontiguousarray(Xs.T)
        .reshape(NKT, P, T)
        .transpose(1, 0, 2)
        .astype(np_bf16)
    )
    xt = np.ascontiguousarray(xt)  # [128, 16, T]

    in_maps = []
    for c in range(N_CORES):
        lo = c * IS
        gw = (
            gate_w[:, :, lo:lo + IS]
            .reshape(E, NKT, P, NMT, P)
            .transpose(0, 3, 2, 1, 4)
            .astype(np_bf16)
        )
        uw = (
            up_w[:, :, lo:lo + IS]
            .reshape(E, NKT, P, NMT, P)
            .transpose(0, 3, 2, 1, 4)
            .astype(np_bf16)
        )
        dw = (
            down_w[:, lo:lo + IS, :]
            .reshape(E, NMT, P, NKT, P)
            .transpose(0, 2, 3, 1, 4)
            .astype(np_bf16)
        )
        in_maps.append(
            {
                "xt": xt,
                "gw": np.ascontiguousarray(gw),
                "uw": np.ascontiguousarray(uw),
                "dw": np.ascontiguousarray(dw),
            }
        )

    nc = _build(counts)
    last_results = run_bass_kernel_spmd(nc, in_maps, list(range(N_CORES)))

    acc = np.zeros((NKT, P, T), np.float32)
    for c in range(N_CORES):
        acc += last_results.results[c]["out"].astype(np.float32)
    out_sorted = acc.reshape(H, T).T  # [T, H] in sorted order
    out = np.empty((T, H), np.float32)
    out[perm] = out_sorted
    return out.reshape(SEQ, BS, H)


# revision 15
# speedup vs baseline: 1.0097x; 1.0097x over previous
"""Llama4 MoE layer (top-1 routing) as an 8-core Trainium2 kernel.

Sharding: tensor-parallel over the intermediate dim I (TP=8). Every core
processes ALL T=8192 tokens, sorted by their routed expert, but holds only a
512-wide I-slice of every expert's gate/up/down weights. Because the sorted
token stream and the expert boundaries are identical on every core, one SPMD
program serves all 8 cores with zero capacity padding — per-core matmul work
is exactly T * (I/8), immune to expert load imbalance (the expert-parallel
alternative pads every core to max_e count(e)). Weight bytes per core are
unchanged vs expert-parallel (each core still holds 1/8 of all weights).

Device math per core (tokens grouped by expert e with c_e tokens each):
    g[I/8, c_e] = gate_w_slice_eᵀ·Xᵀ, u likewise   (K = H, 16 k-tiles)
    a = silu(g) * u                                 (bf16, SBUF-resident)
    y_partial[H, c_e] = down_w_sliceᵀ·a             (K = I/8, 4 k-tiles)
Partial outputs (bf16) are summed across the 8 cores on the host and
scattered back to the original token order. Matmuls in bf16, PSUM f32.

Schedule: phase A runs m-outer (each gate/up weight stripe sweeps all token
chunks) so weight-DMA demand is spread at ~70 GB/s instead of front-loaded.
Input DMAs issue on the sync HWDGE queue in need-order (x of e+1, dw of e,
gate/up of e+1) — DRAM sources are always ready, so the input stream runs a
full expert ahead of the PE. Output rows accumulate per down-m-tile into one
bf16 staging tile and leave as a single DMA per (expert, m), alternating
between the sync and scalar HWDGE queues (trigger cost is ~0.6 µs per DMA
independent of size, so fewer/bigger transfers win).
"""

import numpy as np
import ml_dtypes

import concourse.bass as bass
import concourse.mybir as mybir
import concourse.tile as tile
from concourse import bacc
from concourse.bass_utils import run_bass_kernel_spmd

SEQ, BS, H, I, E = 2048, 4, 2048, 4096, 8
N_CORES = 8
P = 128
T = SEQ * BS          # 8192 tokens
IS = I // N_CORES     # 512-wide I-slice per core
NKT = H // P          # 16 k-tiles for gate/up (K = H)
NMT = IS // P         # 4 m-tiles for gate/up / k-tiles for down
NCHUNK = 512          # max matmul moving-dim / PSUM bank width (f32)

BF16 = mybir.dt.bfloat16
F32 = mybir.dt.float32
np_bf16 = ml_dtypes.bfloat16

# Stash of the last BassKernelResults (exec_time_ns when BASS_TRACE=1).
last_results = None


def _chunks(c, first_small=False):
    """Split c tokens into near-equal chunks of at most NCHUNK."""
    if c <= 0:
        return []
    out = []
    if first_small and c > 384:
        # Small opening chunk so the first matmuls' DMA deps are tiny.
        out.append(128)
        c -= 128
    n = -(-c // NCHUNK)
    base, rem = divmod(c, n)
    out.extend([base + 1] * rem + [base] * (n - rem))
    return out


def _build(counts):
    """Build + compile the per-core TP-over-I program.

    counts: per-expert token counts (global, identical on all cores).
    """
    nc = bacc.Bacc("TRN2", target_bir_lowering=False, debug=False)
    xt_d = nc.dram_tensor("xt", [P, NKT, T], BF16, kind="ExternalInput")
    gw_d = nc.dram_tensor("gw", [E, NMT, P, NKT, P], BF16, kind="ExternalInput")
    uw_d = nc.dram_tensor("uw", [E, NMT, P, NKT, P], BF16, kind="ExternalInput")
    dw_d = nc.dram_tensor("dw", [E, P, NKT, NMT, P], BF16, kind="ExternalInput")
    out_d = nc.dram_tensor("out", [NKT, P, T], BF16, kind="ExternalOutput")

    silu = mybir.ActivationFunctionType.Silu

    # Process experts smallest-first: the first expert's x arrives during the
    # post-reset DMA bandwidth ramp, so give it the smallest working set.
    order = sorted(range(E), key=lambda e: counts[e])

    # Per-expert chunk lists: (global token offset, local offset, size).
    cum = [0] * (E + 1)
    for e in range(E):
        cum[e + 1] = cum[e] + counts[e]
    plans = []
    off = []
    for i, e in enumerate(order):
        ch = []
        loc = 0
        for sz in _chunks(counts[e], first_small=(i == 0)):
            ch.append((cum[e] + loc, loc, sz))
            loc += sz
        plans.append(ch)
        off.append(cum[e])
    ocounts = [counts[e] for e in order]
    amax = max(counts)

    with tile.TileContext(nc) as tc:
        with (
            tc.tile_pool(name="xp", bufs=2) as xp,
            tc.tile_pool(name="wp", bufs=6) as wp,
            tc.tile_pool(name="dp", bufs=2) as dp,
            tc.tile_pool(name="ap", bufs=2) as ap,
            tc.tile_pool(name="sp", bufs=4) as sp,
            tc.tile_pool(name="op", bufs=8) as op,
            tc.tile_pool(name="pp", bufs=8, space="PSUM") as pp,
        ):
            # Warm the PE HAM clock gate with dummy matmuls while the first
            # DMAs are in flight (no data deps — zeros in, discarded out).
            warm = sp.tile([P, NCHUNK], BF16, tag="warm", bufs=1)
            nc.gpsimd.memset(warm[:], 0.0)
            warm_ps = pp.tile([P, NCHUNK], F32, tag="ps")
            for _ in range(32):
                nc.tensor.matmul(
                    warm_ps[:], warm[:, :P], warm[:], start=True, stop=True
                )

            gw_t = {}
            uw_t = {}
            # First-expert startup, in PE need-order: gate/up m0 stripes, the
            # small first x chunk, then the remaining m stripes and x chunks.
            e0 = order[0]
            gw_t[(0, 0)] = wp.tile([P, NKT, P], BF16, tag="gw", name="gw_0_0")
            nc.sync.dma_start(gw_t[(0, 0)][:], gw_d[e0, 0])
            xe0 = xp.tile([P, NKT, amax], BF16, tag="xt", name="xe0")
            goff0, loc0, sz0 = plans[0][0]
            nc.sync.dma_start(
                xe0[:, :, loc0:loc0 + sz0], xt_d[:, :, goff0:goff0 + sz0]
            )
            uw_t[(0, 0)] = wp.tile([P, NKT, P], BF16, tag="uw", name="uw_0_0")
            nc.sync.dma_start(uw_t[(0, 0)][:], uw_d[e0, 0])
            for goff, loc, sz in plans[0][1:]:
                nc.sync.dma_start(
                    xe0[:, :, loc:loc + sz], xt_d[:, :, goff:goff + sz]
                )
            for m in range(1, NMT):
                gw_t[(0, m)] = wp.tile([P, NKT, P], BF16, tag="gw",
                                       name=f"gw_0_{m}")
                nc.sync.dma_start(gw_t[(0, m)][:], gw_d[e0, m])
                uw_t[(0, m)] = wp.tile([P, NKT, P], BF16, tag="uw",
                                       name=f"uw_0_{m}")
                nc.sync.dma_start(uw_t[(0, m)][:], uw_d[e0, m])

            xe = xe0
            for e in range(E):
                oe = order[e]
                chunks = plans[e]
                c_e = ocounts[e]

                # -- input DMAs for the future, in PE need-order --
                if e + 1 < E:
                    xe_next = xp.tile([P, NKT, amax], BF16, tag="xt",
                                      name=f"xe{e + 1}")
                    nc.sync.dma_start(
                        xe_next[:, :, :ocounts[e + 1]],
                        xt_d[:, :, off[e + 1]:off[e + 1] + ocounts[e + 1]],
                    )
                dw_t = dp.tile([P, NKT, NMT, P], BF16, tag="dw")
                nc.sync.dma_start(dw_t[:], dw_d[oe])
                if e + 1 < E:
                    on = order[e + 1]
                    for m in range(NMT):
                        gw_t[(e + 1, m)] = wp.tile(
                            [P, NKT, P], BF16, tag="gw", name=f"gw_{e+1}_{m}")
                        nc.sync.dma_start(gw_t[(e + 1, m)][:], gw_d[on, m])
                        uw_t[(e + 1, m)] = wp.tile(
                            [P, NKT, P], BF16, tag="uw", name=f"uw_{e+1}_{m}")
                        nc.sync.dma_start(uw_t[(e + 1, m)][:], uw_d[on, m])

                act = ap.tile([P, NMT, amax], BF16, tag="act")

                # ---- phase A (m-outer): gate/up + silu*up ----
                for m in range(NMT):
                    for ci, (goff, loc, sz) in enumerate(chunks):
                        psg = pp.tile([P, sz], F32, tag="ps",
                                      name=f"psg{e}_{ci}_{m}")
                        for k in range(NKT):
                            nc.tensor.matmul(
                                psg[:],
                                gw_t[(e, m)][:, k, :],
                                xe[:, k, loc:loc + sz],
                                start=(k == 0),
                                stop=(k == NKT - 1),
                            )
                        psu = pp.tile([P, sz], F32, tag="ps",
                                      name=f"psu{e}_{ci}_{m}")
                        for k in range(NKT):
                            nc.tensor.matmul(
                                psu[:],
                                uw_t[(e, m)][:, k, :],
                                xe[:, k, loc:loc + sz],
                                start=(k == 0),
                                stop=(k == NKT - 1),
                            )
                        sil = sp.tile([P, NCHUNK], F32, tag="sil")
                        nc.scalar.activation(sil[:, :sz], psg[:], silu)
                        nc.vector.tensor_mul(
                            act[:, m, loc:loc + sz], sil[:, :sz], psu[:]
                        )

                # ---- phase B: down partial sums, one out DMA per m ----
                for m in range(NKT):
                    ot = op.tile([P, amax], BF16, tag="ot")
                    for ci, (goff, loc, sz) in enumerate(chunks):
                        psd = pp.tile([P, sz], F32, tag="ps",
                                      name=f"psd{e}_{ci}_{m}")
                        for k in range(NMT):
                            nc.tensor.matmul(
                                psd[:],
                                dw_t[:, m, k, :],
                                act[:, k, loc:loc + sz],
                                start=(k == 0),
                                stop=(k == NMT - 1),
                            )
                        if m % 2 == 0:
                            nc.vector.tensor_copy(ot[:, loc:loc + sz], psd[:])
                        else:
                            nc.scalar.activation(
                                ot[:, loc:loc + sz], psd[:],
                                mybir.ActivationFunctionType.Copy,
                            )
                        if e == E - 1:
                            # Last expert: drain per chunk so the final out
                            # rows leave before the exit barrier, not after
                            # the very last matmul.
                            eng = nc.sync if m % 2 == 0 else nc.scalar
                            eng.dma_start(
                                out_d[m][:, goff:goff + sz],
                                ot[:, loc:loc + sz],
                            )
                    if e < E - 1:
                        eng = nc.sync if m % 2 == 0 else nc.scalar
                        eng.dma_start(
                            out_d[m][:, off[e]:off[e] + c_e], ot[:, :c_e]
                        )

                xe = xe_next if e + 1 < E else None

    nc.compile()
    return nc


def kernel(hidden_states, router_w, gate_w, up_w, down_w):
    global last_results
    X = np.asarray(hidden_states, dtype=np.float32).reshape(-1, H)
    router_w = np.asarray(router_w, dtype=np.float32)
    gate_w = np.asarray(gate_w, dtype=np.float32)
    up_w = np.asarray(up_w, dtype=np.float32)
    down_w = np.asarray(down_w, dtype=np.float32)

    # --- routing (host): top-1 expert per token, mirrored op-for-op on the
    # reference (jnp.einsum + argmax) so near-tied logits resolve identically.
    import jax.numpy as jnp

    logits = jnp.einsum(
        "sbh,he->sbe", np.asarray(hidden_states, dtype=np.float32), router_w
    )
    eid = np.asarray(jnp.argmax(logits, axis=-1)).reshape(-1)  # [T]
    perm = np.argsort(eid, kind="stable")
    counts = np.bincount(eid, minlength=E).tolist()

    Xs = X[perm]  # [T, H] sorted by expert
    xt = (
        np.ascontiguousarray(Xs.T)
        .reshape(NKT, P, T)
        .transpose(1, 0, 2)
        .astype(np_bf16)
    )
    xt = np.ascontiguousarray(xt)  # [128, 16, T]

    in_maps = []
    for c in range(N_CORES):
        lo = c * IS
        gw = (
            gate_w[:, :, lo:lo + IS]
            .reshape(E, NKT, P, NMT, P)
            .transpose(0, 3, 2, 1, 4)
            .astype(np_bf16)
        )
        uw = (
            up_w[:, :, lo:lo + IS]
            .reshape(E, NKT, P, NMT, P)
            .transpose(0, 3, 2, 1, 4)
            .astype(np_bf16)
        )
        dw = (
            down_w[:, lo:lo + IS, :]
            .reshape(E, NMT, P, NKT, P)
            .transpose(0, 2, 3, 1, 4)
            .astype(np_bf16)
        )
        in_maps.append(
            {
                "xt": xt,
                "gw": np.ascontiguousarray(gw),
                "uw": np.ascontiguousarray(uw),
                "dw": np.ascontiguousarray(dw),
            }
        )

    nc = _build(counts)
    last_results = run_bass_kernel_spmd(nc, in_maps, list(range(N_CORES)))

    acc = np.zeros((NKT, P, T), np.float32)
    for c in range(N_CORES):
        acc += last_results.results[c]["out"].astype(np.float32)
    out_sorted = acc.reshape(H, T).T  # [T, H] in sorted order
    out = np.empty((T, H), np.float32)
    out[perm] = out_sorted
    return out.reshape(SEQ, BS, H)
